# revision 3
# baseline (speedup 1.0000x reference)
"""CLAHE (nn_CLAHE) Trainium2 Bass kernel — 8-core SPMD.

Sharding: image rows split 8 ways (512 rows/core); 16x16 tiles never span
cores. mapping_kernel replicated to all cores.

Per-core algorithm (tile-per-partition layout, slabs of 128 tiles):
  u = floor(x*256/255), r = round(x)           (exact fp32 magic-number tricks)
  rank[t,b] = #(u <= b)  via fused is_le+accum  (256 DVE ops/slab)
  hist = diff(rank); clipped = min(hist,4); F = cumsum (log-doubling)
  W[t,b] = (F[b]-F[0]+E*b/256) * 255/(256-cdf_min) * sigmoid(mk[b])
  out = W[t, r]  via per-bin masked multiply-accumulate
"""
import numpy as np
from contextlib import ExitStack

import concourse.bass as bass
import concourse.tile as tile
from concourse import bacc, mybir
from concourse.bass_utils import run_bass_kernel_spmd

f32 = mybir.dt.float32
i32 = mybir.dt.int32
Alu = mybir.AluOpType
Act = mybir.ActivationFunctionType

H = W_IMG = 4096
N_CORES = 8
ROWS = H // N_CORES  # 512 rows per core
COLS = W_IMG
N_BINS = 256
TILE = 16
PX = TILE * TILE


def _emit_clahe(ctx, tc, y_ap, x_ap, mk_ap, rows, cols):
    nc = tc.nc
    n_tiles = (rows // TILE) * (cols // TILE)
    n_slabs = n_tiles // 128

    xv = x_ap.rearrange("(tr p) (tc q) -> tr tc p q", p=TILE, q=TILE)
    yv = y_ap.rearrange("(tr p) (tc q) -> tr tc p q", p=TILE, q=TILE)

    const_pool = ctx.enter_context(tc.tile_pool(name="const", bufs=1))
    io_pool = ctx.enter_context(tc.tile_pool(name="io", bufs=3))
    work_pool = ctx.enter_context(tc.tile_pool(name="work", bufs=2))

    mk_row = const_pool.tile([1, N_BINS], f32, tag="mkrow")
    nc.sync.dma_start(mk_row[:], mk_ap[:])
    sig = const_pool.tile([128, N_BINS], f32, tag="sig")
    nc.gpsimd.partition_broadcast(sig[:], mk_row[:], channels=128)
    nc.scalar.activation(sig[:], sig[:], Act.Sigmoid)

    bgrid_i = const_pool.tile([128, N_BINS], i32, tag="bgridi")
    nc.gpsimd.iota(bgrid_i[:], pattern=[[1, N_BINS]], base=0, channel_multiplier=0)
    bgrid = const_pool.tile([128, N_BINS], f32, tag="bgrid")
    nc.vector.tensor_copy(bgrid[:], bgrid_i[:])
    nc.vector.tensor_scalar(bgrid[:], bgrid[:], 1.0 / N_BINS, None, Alu.mult)

    MAGIC = float(2 ** 23)

    for s in range(n_slabs):
        X = io_pool.tile([128, PX], f32, tag="X")
        tr, tc0 = divmod(s * 128, cols // TILE)
        nc.sync.dma_start(X[:], xv[tr, tc0:tc0 + 128])

        y_t = work_pool.tile([128, PX], f32, tag="y_t")
        nc.scalar.activation(y_t[:], X[:], Act.Copy, scale=float(N_BINS / 255.0))
        u = work_pool.tile([128, PX], f32, tag="u")
        nc.vector.tensor_scalar(u[:], y_t[:], MAGIC, -MAGIC, Alu.add, Alu.add)
        frac = work_pool.tile([128, PX], f32, tag="frac")
        nc.vector.tensor_tensor(frac[:], u[:], y_t[:], Alu.is_gt)
        nc.vector.tensor_tensor(u[:], u[:], frac[:], Alu.subtract)

        r = work_pool.tile([128, PX], f32, tag="r")
        nc.vector.tensor_scalar(r[:], X[:], MAGIC, -MAGIC, Alu.add, Alu.add)

        # rank pass in bf16 (u integer-valued <= 255: bf16-exact; 2x DVE mode)
        bf16 = mybir.dt.bfloat16
        u_bf = work_pool.tile([128, PX], bf16, tag="u_bf")
        nc.vector.tensor_copy(u_bf[:], u[:])
        rank = work_pool.tile([128, N_BINS], f32, tag="rank")
        scratch = work_pool.tile([128, PX], bf16, tag="scratch")
        for b in range(N_BINS):
            nc.vector.tensor_scalar(scratch[:], u_bf[:], float(b), None, Alu.is_le,
                                    Alu.add, accum_out=rank[:, b:b + 1])

        m = work_pool.tile([128, N_BINS], f32, tag="m")
        nc.vector.tensor_copy(m[:, 0:1], rank[:, 0:1])
        nc.vector.tensor_tensor(m[:, 1:N_BINS], rank[:, 1:N_BINS], rank[:, 0:N_BINS - 1], Alu.subtract)
        nc.vector.tensor_scalar(m[:], m[:], 4.0, None, Alu.min)

        Fa = work_pool.tile([128, N_BINS], f32, tag="Fa")
        Fb = work_pool.tile([128, N_BINS], f32, tag="Fb")
        nc.vector.tensor_copy(Fa[:], m[:])
        cur, nxt = Fa, Fb
        d = 1
        while d < N_BINS:
            nc.vector.tensor_copy(nxt[:, 0:d], cur[:, 0:d])
            nc.vector.tensor_tensor(nxt[:, d:N_BINS], cur[:, d:N_BINS], cur[:, 0:N_BINS - d], Alu.add)
            cur, nxt = nxt, cur
            d *= 2
        F = cur

        E = work_pool.tile([128, 1], f32, tag="E")
        nc.vector.tensor_scalar(E[:], F[:, N_BINS - 1:N_BINS], -1.0, float(N_BINS), Alu.mult, Alu.add)
        cm = work_pool.tile([128, 1], f32, tag="cm")
        nc.vector.tensor_scalar(cm[:], E[:], 1.0 / N_BINS, None, Alu.mult)
        nc.vector.tensor_tensor(cm[:], cm[:], F[:, 0:1], Alu.add)
        gam = work_pool.tile([128, 1], f32, tag="gam")
        nc.vector.tensor_scalar(gam[:], cm[:], -1.0, float(N_BINS), Alu.mult, Alu.add)
        nc.vector.tensor_scalar(gam[:], gam[:], 1e-7, None, Alu.max)
        nc.vector.reciprocal(gam[:], gam[:])
        nc.vector.tensor_scalar(gam[:], gam[:], 255.0, None, Alu.mult)

        W = work_pool.tile([128, N_BINS], f32, tag="W")
        nc.vector.tensor_scalar(W[:], F[:], F[:, 0:1], None, Alu.subtract)
        Egrid = nxt
        nc.vector.tensor_scalar(Egrid[:], bgrid[:], E[:], None, Alu.mult)
        nc.vector.tensor_tensor(W[:], W[:], Egrid[:], Alu.add)
        nc.vector.tensor_scalar(W[:], W[:], gam[:], None, Alu.mult)
        nc.vector.tensor_tensor(W[:], W[:], sig[:], Alu.mult)

        acc = io_pool.tile([128, PX], f32, tag="acc")
        sel = work_pool.tile([128, PX], f32, tag="sel")
        nc.vector.memset(acc[:], 0.0)
        for b in range(N_BINS):
            nc.vector.tensor_scalar(sel[:], r[:], float(b), W[:, b:b + 1], Alu.is_equal, Alu.mult)
            nc.vector.tensor_tensor(acc[:], acc[:], sel[:], Alu.add)

        nc.sync.dma_start(yv[tr, tc0:tc0 + 128], acc[:])


_CACHED_NC = None


def _build():
    global _CACHED_NC
    if _CACHED_NC is not None:
        return _CACHED_NC
    nc = bacc.Bacc("TRN2", target_bir_lowering=False, debug=False,
                   enable_asserts=False, num_devices=N_CORES)
    x = nc.dram_tensor("x", [ROWS, COLS], f32, kind="ExternalInput").ap()
    mk = nc.dram_tensor("mk", [1, N_BINS], f32, kind="ExternalInput").ap()
    y = nc.dram_tensor("y", [ROWS, COLS], f32, kind="ExternalOutput").ap()
    with tile.TileContext(nc) as tc:
        with ExitStack() as ctx:
            _emit_clahe(ctx, tc, y, x, mk, ROWS, COLS)
    nc.compile()
    _CACHED_NC = nc
    return nc


def kernel(inputs: np.ndarray, mapping_kernel: np.ndarray) -> np.ndarray:
    x = np.ascontiguousarray(np.asarray(inputs, dtype=np.float32)[:, :, 0])
    mk = np.ascontiguousarray(np.asarray(mapping_kernel, dtype=np.float32)).reshape(1, N_BINS)
    nc = _build()
    in_maps = [{"x": x[ROWS * c: ROWS * (c + 1)], "mk": mk} for c in range(N_CORES)]
    res = run_bass_kernel_spmd(nc, in_maps, core_ids=list(range(N_CORES)))
    out = np.concatenate([res.results[c]["y"] for c in range(N_CORES)], axis=0)
    return out[:, :, None].astype(np.float32)
